# revision 1
# baseline (speedup 1.0000x reference)
"""Trainium2 Bass kernel for a 2-layer LSTM decoder (B=128, T=32, F=2048,
E=512, H=1024, V=10000), data-parallel over batch across 8 NeuronCores.

Per-core plan (batch shard BL=16):
 - All matmuls use mode "activations stationary / weights streaming":
   out[batch, feat] = lhsT(=acts.T [K, BL]).T @ rhs(=W.T [K, N]).
   Stream cost is independent of batch size, so DP costs nothing on the
   sequential recurrence while sharding everything else 8x.
 - fp16 matmul operands (1 cycle/row on PE vs 4 for fp32), fp32 PSUM and
   fp32 cell state / gate activations.
 - whh0.T and whh1.T are SBUF-resident across all 32 steps; wih1.T is
   streamed from HBM each step in quarter-G pieces; wih0 is pre-applied to
   all timesteps at once (X0 = emb @ wih0.T + b0) through a DRAM scratch.
 - Biases are folded in with K=1 ones-row matmuls; gate order is
   host-permuted to [i, f, o, g]; init_h/init_c weights host-permuted to
   layer-major so h0/c0 split into layers by column range.
"""

import numpy as np

import concourse.bass as bass
import concourse.mybir as mybir
from concourse import bacc
from concourse.bass import MemorySpace
from concourse.bass_utils import run_bass_kernel_spmd
from concourse.masks import make_identity
from concourse.tile import TileContext

P = 128
NCORES = 8
B, T, F, E, H, L, V = 128, 32, 2048, 512, 1024, 2, 10000
G = 4 * H
BL = B // NCORES          # 16 batch rows per core
TB = T * BL               # 512 (t, b) rows per core
HL = H * L
KF, KE, KH = F // P, E // P, H // P      # 16, 4, 8
NB = G // 512             # 8 psum 512-col chunks across G
F16 = mybir.dt.float16
F32 = mybir.dt.float32

_cache = {}


def _build_nc(phases="ABCDE", fake_gather=False, d_mode=0):
    nc = bacc.Bacc("TRN2", target_bir_lowering=False, debug=False,
                   enable_asserts=False, num_devices=NCORES)

    dram = {}

    def din(name, shape, dt=F16):
        dram[name] = nc.dram_tensor(name, shape, dt, kind="ExternalInput").ap()
        return dram[name]

    featT = din("featT", [F, BL])
    emb_idx = din("emb_idx", [TB, 1], mybir.dt.int32)
    table = din("table", [V, E])
    init_hw = din("init_hw", [F, HL])
    init_cw = din("init_cw", [F, HL])
    init_hb = din("init_hb", [1, HL])
    init_cb = din("init_cb", [1, HL])
    wih0T = din("wih0T", [E, G])
    whh0T = din("whh0T", [H, G])
    wih1T = din("wih1T", [G, 1024])  # quarter-major: row q*H + h
    whh1T = din("whh1T", [H, G])
    bsum0 = din("bsum0", [1, G])
    bsum1 = din("bsum1", [1, G])
    fcwT = din("fcwT", [H, V])
    fcb_rep = din("fcb_rep", [P, V], F32)

    out = nc.dram_tensor("out", [TB, V], F32, kind="ExternalOutput").ap()
    x0buf = nc.dram_tensor("x0buf", [TB, G], F16, kind="Internal").ap()

    # DRAM views with the partition dim split out: row r = k*P + p
    featT_v = featT.rearrange("(k p) b -> p k b", p=P)
    init_hw_v = init_hw.rearrange("(k p) n -> p k n", p=P)
    init_cw_v = init_cw.rearrange("(k p) n -> p k n", p=P)
    wih0T_v = wih0T.rearrange("(k p) g -> p k g", p=P)
    whh0T_v = whh0T.rearrange("(k p) g -> p k g", p=P)
    wih1T_v = wih1T.rearrange("(q k p) n -> p q k n", q=4, p=P)
    whh1T_v = whh1T.rearrange("(k p) g -> p k g", p=P)
    fcwT_v = fcwT.rearrange("(k p) v -> p k v", p=P)
    idx_v = emb_idx.rearrange("(g p) one -> p g one", p=P)

    SIG = mybir.ActivationFunctionType.Sigmoid
    TANH = mybir.ActivationFunctionType.Tanh

    with TileContext(nc) as tc:
        with tc.tile_pool(name="const", bufs=1) as constp, \
             tc.tile_pool(name="resident", bufs=1) as resp, \
             tc.tile_pool(name="state", bufs=1) as statep, \
             tc.tile_pool(name="h0t", bufs=2) as h0tp, \
             tc.tile_pool(name="ys", bufs=1) as ysp:

            id128 = constp.tile([P, P], F16)
            make_identity(nc, id128)
            id16 = constp.tile([BL, BL], F16)
            make_identity(nc, id16)
            ones16 = constp.tile([1, BL], F16)
            nc.gpsimd.memset(ones16, 1.0)
            ones128 = constp.tile([1, P], F16)
            nc.gpsimd.memset(ones128, 1.0)
            bsum1_s = constp.tile([1, G], F16)
            nc.sync.dma_start(bsum1_s, bsum1)

            # SBUF-resident recurrent weights (64 KB/partition each)
            whh0_s = resp.tile([P, KH, G], F16)
            nc.sync.dma_start(whh0_s, whh0T_v)
            whh1_s = resp.tile([P, KH, G], F16)
            nc.sync.dma_start(whh1_s, whh1T_v)

            # Long-lived state
            c0_s = statep.tile([BL, H], F32)
            c1_s = statep.tile([BL, H], F32)
            h1T0_s = statep.tile([P, KH, BL], F16)   # t=0 layer-1 h.T
            # all layer-1 h.T outputs (feeds both recurrence and FC)
            ysT = ysp.tile([P, KH, T, BL], F16)

            # ---------------- Phases A-C scratch ---------------------------
            abc_pool = tc.alloc_tile_pool(name="embT", bufs=1)
            embT_s = abc_pool.tile([P, KE, TB], F16)  # transposed embeddings

            # ---------------- Phase A: embedding gather + transpose -------
            with tc.tile_pool(name="embp", bufs=2) as embp, \
                 tc.tile_pool(name="embpsum", bufs=2, space="PSUM") as embps:
                for g in range(TB // P):
                    if "A" not in phases:
                        break
                    rows = embp.tile([P, E], F16, tag="rows")
                    if fake_gather:
                        nc.sync.dma_start(rows, table[g * P : (g + 1) * P, :])
                    else:
                        idx_t = embp.tile([P, 1, 1], mybir.dt.int32, tag="idx")
                        nc.sync.dma_start(idx_t, idx_v[:, g : g + 1, :])
                        nc.gpsimd.indirect_dma_start(
                            out=rows[:],
                            out_offset=None,
                            in_=table[:],
                            in_offset=bass.IndirectOffsetOnAxis(
                                ap=idx_t[:, 0, :], axis=0
                            ),
                        )
                    pt = embps.tile([P, KE, P], F16, tag="pt")
                    for ke in range(KE):
                        nc.tensor.transpose(
                            pt[:, ke, :], rows[:, ke * P : (ke + 1) * P], id128
                        )
                    nc.vector.tensor_copy(
                        embT_s[:, :, g * P : (g + 1) * P], pt
                    )

            # ---------------- Phase B: h0/c0 init --------------------------
            with tc.tile_pool(name="initw", bufs=3) as initwp, \
                 tc.tile_pool(name="initsb", bufs=2) as initsb, \
                 tc.tile_pool(name="initpsum", bufs=1, space="PSUM") as initps:
                ihb_s = initsb.tile([1, HL], F16, tag="ib0")
                nc.sync.dma_start(ihb_s, init_hb)
                icb_s = initsb.tile([1, HL], F16, tag="ib1")
                nc.sync.dma_start(icb_s, init_cb)
                featT_s = initsb.tile([P, KF, BL], F16, tag="ft")
                nc.sync.dma_start(featT_s, featT_v)
                for which, (wv, bias_s) in enumerate(
                    ((init_hw_v, ihb_s), (init_cw_v, icb_s))
                ):
                    if "B" not in phases:
                        break
                    ps = initps.tile([BL, 4, 512], F32, tag="initps")
                    for k in range(KF):
                        wc = initwp.tile([P, 1, HL], F16, tag="iwc")
                        nc.sync.dma_start(wc, wv[:, k : k + 1, :])
                        for n in range(4):
                            nc.tensor.matmul(
                                ps[:, n, :],
                                featT_s[:, k, :],
                                wc[:, 0, n * 512 : (n + 1) * 512],
                                start=(k == 0),
                                stop=False,
                            )
                    for n in range(4):
                        nc.tensor.matmul(
                            ps[:, n, :],
                            ones16,
                            bias_s[:, n * 512 : (n + 1) * 512],
                            start=False,
                            stop=True,
                        )
                    if which == 0:
                        # h0: layer-major columns; cast to fp16, transpose
                        hh = initsb.tile([BL, HL], F16, tag="hh")
                        nc.vector.tensor_copy(hh, ps)
                        with tc.tile_pool(name="trps", bufs=2,
                                          space="PSUM") as trps:
                            for lay in range(L):
                                pt = trps.tile([P, KH, BL], F16, tag="pt")
                                for j in range(KH):
                                    nc.tensor.transpose(
                                        pt[:, j, :],
                                        hh[:, lay * H + j * P : lay * H + (j + 1) * P],
                                        id16,
                                    )
                                if lay == 0:
                                    h0T = h0tp.tile([P, KH, BL], F16, tag="h0T")
                                    nc.vector.tensor_copy(h0T, pt)
                                else:
                                    nc.vector.tensor_copy(h1T0_s, pt)
                    else:
                        nc.vector.tensor_copy(c0_s, ps[:, 0:2, :])
                        nc.vector.tensor_copy(c1_s, ps[:, 2:4, :])

            # ---------------- Phase C: X0 = emb @ wih0.T + b0 --------------
            with tc.tile_pool(name="wih0p", bufs=1) as wih0p, \
                 tc.tile_pool(name="x0sb", bufs=2) as x0sb, \
                 tc.tile_pool(name="x0psum", bufs=2, space="PSUM") as x0ps, \
                 tc.tile_pool(name="b0p", bufs=1) as b0p:
                wih0_s = wih0p.tile([P, KE, G], F16)
                nc.sync.dma_start(wih0_s, wih0T_v)
                bsum0_s = b0p.tile([1, G], F16)
                nc.sync.dma_start(bsum0_s, bsum0)
                for m in range(TB // P):
                    if "C" not in phases:
                        break
                    for half in range(2):
                        ps = x0ps.tile([P, 4, 512], F32, tag="x0ps")
                        for k in range(KE):
                            for n in range(4):
                                col = half * 2048 + n * 512
                                nc.tensor.matmul(
                                    ps[:, n, :],
                                    embT_s[:, k, m * P : (m + 1) * P],
                                    wih0_s[:, k, col : col + 512],
                                    start=(k == 0),
                                    stop=False,
                                )
                        for n in range(4):
                            col = half * 2048 + n * 512
                            nc.tensor.matmul(
                                ps[:, n, :],
                                ones128,
                                bsum0_s[:, col : col + 512],
                                start=False,
                                stop=True,
                            )
                        xs = x0sb.tile([P, 2048], F16, tag="xs")
                        nc.vector.tensor_copy(xs, ps)
                        nc.sync.dma_start(
                            x0buf[m * P : (m + 1) * P,
                                  half * 2048 : (half + 1) * 2048],
                            xs,
                        )
            abc_pool.release()

            # ---------------- Phase D: recurrence --------------------------
            # Software-pipelined emission order per step keeps the PE fed:
            #   [L0 whh0+X0 MMs] [h1(t-1) transposes] [L1 whh1 half]
            #   [h0(t) transposes] [L1 wih1 half + bias] ...
            # so layer-1 matmuls that depend only on old state cover the
            # latency of layer-0's ACT/DVE elementwise chain.
            with tc.tile_pool(name="wih1p", bufs=2) as wih1p, \
                 tc.tile_pool(name="x0tp", bufs=1) as x0tp, \
                 tc.tile_pool(name="gact", bufs=1) as gact, \
                 tc.tile_pool(name="hsb", bufs=2) as hsbp, \
                 tc.tile_pool(name="gpsum", bufs=3, space="PSUM") as gps, \
                 tc.tile_pool(name="trpsum", bufs=2, space="PSUM") as trps:

                def drain_gate(ps, gp, lay, c_s, acc):
                    """ACT drain of one gate piece; returns via acc dict."""
                    if gp == 3:
                        tg = gact.tile([BL, 1024], F32, tag="tg")
                        nc.scalar.activation(tg, ps, TANH)
                        acc["tanh_g"] = tg
                    elif gp == 1:
                        sig_f = gact.tile([BL, 1024], F32, tag="tg")
                        nc.scalar.activation(sig_f, ps, SIG)
                        nc.vector.tensor_mul(c_s, sig_f, c_s)
                    else:
                        sg = gact.tile([BL, 1024], F32, tag=f"sig{gp}")
                        nc.scalar.activation(sg, ps, SIG)
                        acc["sig_i" if gp == 0 else "sig_o"] = sg

                def elementwise_tail(acc, c_s):
                    """c += sig_i*tanh_g; h = sig_o*tanh(c) -> fp16 tile."""
                    tanh_g, sig_i, sig_o = acc["tanh_g"], acc["sig_i"], acc["sig_o"]
                    nc.vector.tensor_mul(tanh_g, sig_i, tanh_g)
                    nc.vector.tensor_add(c_s, c_s, tanh_g)
                    tc_t = gact.tile([BL, H], F32, tag="tg")
                    nc.scalar.activation(tc_t, c_s, TANH)
                    h_sb = hsbp.tile([BL, H], F16, tag="hsb")
                    nc.vector.tensor_mul(h_sb, sig_o, tc_t)
                    return h_sb

                def transpose_h(h_sb):
                    pt = trps.tile([P, KH, BL], F16, tag="pt")
                    for j in range(KH):
                        nc.tensor.transpose(
                            pt[:, j, :], h_sb[:, j * P : (j + 1) * P], id16
                        )
                    return pt

                h0T_prev = h0T
                h1_sb_prev = None
                for t in range(T):
                    if "D" not in phases:
                        break
                    # ---- L0: gates0 = whh0 @ h0 + X0[t] --------------------
                    acc0 = {}
                    x0t = None
                    for gp in range(4):
                        if gp % 2 == 0:
                            x0t = x0tp.tile([BL, 2048], F16, tag="x0t")
                            nc.sync.dma_start(
                                x0t,
                                x0buf[t * BL : (t + 1) * BL,
                                      (gp // 2) * 2048 : (gp // 2 + 1) * 2048],
                            )
                        ps = gps.tile([BL, 1024], F32, tag="gp")
                        for k in range(KH):
                            for nn in range(2):
                                col = gp * 1024 + nn * 512
                                nc.tensor.matmul(
                                    ps[:, nn * 512 : (nn + 1) * 512],
                                    h0T_prev[:, k, :],
                                    whh0_s[:, k, col : col + 512],
                                    start=(k == 0),
                                    stop=False,
                                )
                        for nn in range(2):
                            hcol = (gp % 2) * 1024 + nn * 512
                            nc.tensor.matmul(
                                ps[:, nn * 512 : (nn + 1) * 512],
                                id16,
                                x0t[:, hcol : hcol + 512],
                                start=False,
                                stop=True,
                            )
                        drain_gate(ps, gp, 0, c0_s, acc0)

                    # ---- h1(t-1) transposes -> ysT[t-1] --------------------
                    if t > 0:
                        pt = transpose_h(h1_sb_prev)
                        nc.vector.tensor_copy(ysT[:, :, t - 1, :], pt)

                    # ---- wih1 quarter prefetches ---------------------------
                    wqs = {}
                    for gp in range(4):
                        wq = wih1p.tile([P, KH, 1024], F16, tag="wq")
                        nc.sync.dma_start(wq, wih1T_v[:, gp, :, :])
                        wqs[gp] = wq

                    # ---- L1 halves: whh1 first (old state), wih1 second ----
                    l1_ps = {}
                    acc1 = {}
                    h0T_new = None
                    for half in range(2):
                        for gp in (2 * half, 2 * half + 1):
                            ps = gps.tile([BL, 1024], F32, tag="gp")
                            l1_ps[gp] = ps
                            for k in range(KH):
                                stat = (
                                    h1T0_s[:, k, :] if t == 0
                                    else ysT[:, k, t - 1, :]
                                )
                                for nn in range(2):
                                    col = gp * 1024 + nn * 512
                                    nc.tensor.matmul(
                                        ps[:, nn * 512 : (nn + 1) * 512],
                                        stat,
                                        whh1_s[:, k, col : col + 512],
                                        start=(k == 0),
                                        stop=False,
                                    )
                        if half == 0:
                            # layer-0 elementwise tail + h0 transposes land
                            # here, covered by the whh1 matmuls above
                            h0_sb = elementwise_tail(acc0, c0_s)
                            pt0 = transpose_h(h0_sb)
                            h0T_new = h0tp.tile([P, KH, BL], F16, tag="h0T")
                            nc.vector.tensor_copy(h0T_new, pt0)
                        for gp in (2 * half, 2 * half + 1):
                            ps = l1_ps[gp]
                            wq = wqs[gp]
                            for k in range(KH):
                                for nn in range(2):
                                    nc.tensor.matmul(
                                        ps[:, nn * 512 : (nn + 1) * 512],
                                        h0T_new[:, k, :],
                                        wq[:, k, nn * 512 : (nn + 1) * 512],
                                        start=False,
                                        stop=False,
                                    )
                            for nn in range(2):
                                col = gp * 1024 + nn * 512
                                nc.tensor.matmul(
                                    ps[:, nn * 512 : (nn + 1) * 512],
                                    ones16,
                                    bsum1_s[:, col : col + 512],
                                    start=False,
                                    stop=True,
                                )
                            drain_gate(ps, gp, 1, c1_s, acc1)

                    h1_sb_prev = elementwise_tail(acc1, c1_s)
                    h0T_prev = h0T_new

                if "D" in phases:
                    pt = transpose_h(h1_sb_prev)
                    nc.vector.tensor_copy(ysT[:, :, T - 1, :], pt)

            # ---------------- Phase E: FC over vocab -----------------------
            with tc.tile_pool(name="fcw", bufs=3) as fcwp, \
                 tc.tile_pool(name="fcb", bufs=2) as fcbp, \
                 tc.tile_pool(name="fcout", bufs=3) as fcoutp, \
                 tc.tile_pool(name="fcpsum", bufs=4, space="PSUM") as fcps:
                nvt = (V + 511) // 512 if "E" in phases else 0
                for vt in range(nvt):
                    w = min(512, V - vt * 512)
                    fcw_t = fcwp.tile([P, KH, 512], F16, tag="fcw")
                    nc.sync.dma_start(
                        fcw_t[:, :, :w], fcwT_v[:, :, vt * 512 : vt * 512 + w]
                    )
                    fcb_t = fcbp.tile([P, 512], F32, tag="fcb")
                    nc.sync.dma_start(
                        fcb_t[:, :w], fcb_rep[:, vt * 512 : vt * 512 + w]
                    )
                    tpm = P // BL  # timesteps per 128-row output chunk
                    for m in range(TB // P):
                        ps = fcps.tile([P, 512], F32, tag="fcps")
                        for k in range(KH):
                            nc.tensor.matmul(
                                ps[:, :w],
                                ysT[:, k, m * tpm : (m + 1) * tpm, :],
                                fcw_t[:, k, :w],
                                start=(k == 0),
                                stop=(k == KH - 1),
                            )
                        ot = fcoutp.tile([P, 512], F32, tag="fcout")
                        nc.vector.tensor_add(ot[:, :w], ps[:, :w], fcb_t[:, :w])
                        nc.sync.dma_start(
                            out[m * P : (m + 1) * P, vt * 512 : vt * 512 + w],
                            ot[:, :w],
                        )

    nc.finalize()
    return nc


def _get_compiled():
    if "nc" not in _cache:
        _cache["nc"] = _build_nc()
    return _cache["nc"]


def _prep_inputs(features, captions, embed_table, init_h_w, init_h_b,
                 init_c_w, init_c_b, w_ih0, w_hh0, b_ih0, b_hh0,
                 w_ih1, w_hh1, b_ih1, b_hh1, fc_w, fc_b):
    f16 = lambda x: np.ascontiguousarray(np.asarray(x), dtype=np.float32).astype(np.float16)
    f32 = lambda x: np.ascontiguousarray(np.asarray(x), dtype=np.float32)

    gperm = [0, 1, 3, 2]  # i, f, g, o -> i, f, o, g

    def gate_permute_T(wmat):
        wmat = np.asarray(wmat, dtype=np.float32)
        k = wmat.shape[1]
        return np.ascontiguousarray(
            wmat.reshape(4, H, k)[gperm].reshape(G, k).T
        ).astype(np.float16)

    def gate_permute_b(b1, b2):
        s = (np.asarray(b1, np.float32) + np.asarray(b2, np.float32))
        return s.reshape(4, H)[gperm].reshape(1, G).astype(np.float16)

    def init_permute_T(wmat):
        # rows r = h*L + l  ->  layer-major rows l*H + h, then transpose
        wmat = np.asarray(wmat, dtype=np.float32)
        return np.ascontiguousarray(
            wmat.reshape(H, L, F).transpose(1, 0, 2).reshape(HL, F).T
        ).astype(np.float16)

    def init_permute_b(bvec):
        bvec = np.asarray(bvec, dtype=np.float32)
        return bvec.reshape(H, L).T.reshape(1, HL).astype(np.float16)

    shared = {
        "table": f16(embed_table),
        "init_hw": init_permute_T(init_h_w),
        "init_cw": init_permute_T(init_c_w),
        "init_hb": init_permute_b(init_h_b),
        "init_cb": init_permute_b(init_c_b),
        "wih0T": gate_permute_T(w_ih0),
        "whh0T": gate_permute_T(w_hh0),
        "wih1T": np.ascontiguousarray(
            gate_permute_T(w_ih1).reshape(H, 4, 1024)
            .transpose(1, 0, 2).reshape(G, 1024)
        ),
        "whh1T": gate_permute_T(w_hh1),
        "bsum0": gate_permute_b(b_ih0, b_hh0),
        "bsum1": gate_permute_b(b_ih1, b_hh1),
        "fcwT": np.ascontiguousarray(
            np.asarray(fc_w, dtype=np.float32).T
        ).astype(np.float16),
        "fcb_rep": np.ascontiguousarray(
            np.broadcast_to(np.asarray(fc_b, np.float32), (P, V))
        ),
    }

    features = np.asarray(features, dtype=np.float32)
    captions = np.asarray(captions).astype(np.int32)

    in_maps = []
    for c in range(NCORES):
        bsl = slice(c * BL, (c + 1) * BL)
        m = dict(shared)
        m["featT"] = np.ascontiguousarray(features[bsl].T).astype(np.float16)
        # row r = t*BL + b  ->  captions[b_global, t]
        m["emb_idx"] = np.ascontiguousarray(
            captions[bsl].T.reshape(TB, 1)
        )
        in_maps.append(m)
    return in_maps


last_results = None


def kernel(**inputs) -> np.ndarray:
    global last_results
    nc = _get_compiled()
    in_maps = _prep_inputs(**inputs)
    res = run_bass_kernel_spmd(nc, in_maps, core_ids=list(range(NCORES)))
    last_results = res
    parts = [res.results[c]["out"].reshape(T, BL, V) for c in range(NCORES)]
    return np.concatenate(parts, axis=1)



# revision 3
# speedup vs baseline: 1.0521x; 1.0521x over previous
"""Trainium2 Bass kernel for a 2-layer LSTM decoder (B=128, T=32, F=2048,
E=512, H=1024, V=10000), tensor-parallel over the hidden dim across 8
NeuronCores.

Sharding: core c owns hidden slice [c*128, (c+1)*128) of BOTH layers (gates
i,f,o,g for that slice = 512 gate rows per weight matrix) and vocab slice
[c*1250, (c+1)*1250) of the FC head. Full batch B=128 on every core, so
every recurrence matmul runs at full 128-wide PE utilization; the per-step
cost is one 64KB AllGather of the {h0T | h1T} slice pair (DRAM-bounce
collective, ~7us ncfw floor) whose latency is hidden behind the FC matmuls
of older timesteps.

All gate math is "orientation B" (transposed): gatesT[g, b] tiles with the
gate index on partitions, so h-slices come out of the elementwise tail
already transposed for the next step's lhsT and for the AllGather, and the
gate biases fold into the ACT bias operand (no bias matmuls).

AG payload rows are 512B ({h0T | h1T} side by side per partition) so the
readback is 512B-descriptor DMA, and a small batch of throwaway matmuls at
the end of each iteration keeps the PE HAM clock-gate at 2.4 GHz across the
collective wait.

Per-step dependency chain (iteration t emits):
  readback R of AG_{t-1}={h0(t-1), h1(t-2)} -> L0(t) -> h0T(t)
  L1(t-1) from R -> h1T(t-1);  AG_t = {h0T(t), h1T(t-1)}
  FC(t-2) from R part1 (per-step PE filler that covers AG latency)
"""

import numpy as np

import concourse.bass as bass
import concourse.mybir as mybir
from concourse import bacc
from concourse.bass_utils import run_bass_kernel_spmd
from concourse.masks import make_identity
from concourse.tile import TileContext

P = 128
NCORES = 8
B, T, F, E, H, L, V = 128, 32, 2048, 512, 1024, 2, 10000
G = 4 * H
TB = T * B                 # 4096 (t, b) rows
Hc = H // NCORES           # 128 hidden units per core
Gc = 4 * Hc                # 512 local gate rows
Vc = V // NCORES           # 1250 vocab cols per core
KE, KF, KH = E // P, F // P, H // P   # 4, 16, 8
NDUMMY = 40                # PE warm-keeper matmuls per iteration
F16 = mybir.dt.float16
F32 = mybir.dt.float32

_cache = {}

SIG = mybir.ActivationFunctionType.Sigmoid
TANH = mybir.ActivationFunctionType.Tanh


def _build_nc():
    nc = bacc.Bacc("TRN2", target_bir_lowering=False, debug=False,
                   enable_asserts=False, num_devices=NCORES)

    def din(name, shape, dt=F16):
        return nc.dram_tensor(name, shape, dt, kind="ExternalInput").ap()

    featT = din("featT", [F, B])
    emb_idx = din("emb_idx", [TB, 1], mybir.dt.int32)
    table = din("table", [V, E])
    initw = din("initw", [F, 4 * P])      # cols: h_l0 | h_l1 | c_l0 | c_l1
    initbT = din("initbT", [P, 4], F32)
    wih0T = din("wih0T", [E, Gc])
    whh0T = din("whh0T", [H, Gc])
    wih1T = din("wih1T", [H, Gc])
    whh1T = din("whh1T", [H, Gc])
    b0T = din("b0T", [P, 4], F32)
    b1row = din("b1row", [1, Gc])
    fcwT = din("fcwT", [H, Vc])
    fcb_rep = din("fcb_rep", [P, Vc], F32)

    out = nc.dram_tensor("out", [TB, Vc], F32, kind="ExternalOutput").ap()

    featT_v = featT.rearrange("(k p) b -> p k b", p=P)
    initw_v = initw.rearrange("(k p) n -> p k n", p=P)
    wih0T_v = wih0T.rearrange("(k p) g -> p k g", p=P)
    whh0T_v = whh0T.rearrange("(k p) g -> p k g", p=P)
    wih1T_v = wih1T.rearrange("(k p) g -> p k g", p=P)
    whh1T_v = whh1T.rearrange("(k p) g -> p k g", p=P)
    fcwT_v = fcwT.rearrange("(k p) v -> p k v", p=P)
    idx_v = emb_idx.rearrange("(g p) one -> p g one", p=P)

    RG = [list(range(NCORES))]

    with TileContext(nc) as tc:
        if True:
            constp = tc.alloc_tile_pool(name="const", bufs=1)
            wresp = tc.alloc_tile_pool(name="wres", bufs=1)
            statep = tc.alloc_tile_pool(name="state", bufs=1)
            x0p = tc.alloc_tile_pool(name="x0", bufs=1)
            rowsp = tc.alloc_tile_pool(name="rows", bufs=32)
            rbp = tc.alloc_tile_pool(name="rb", bufs=2)
            hpairp = tc.alloc_tile_pool(name="hpair", bufs=2)
            gactp = tc.alloc_tile_pool(name="gact", bufs=2)
            fcoutp = tc.alloc_tile_pool(name="fcout", bufs=3)
            aginp = tc.alloc_tile_pool(name="agin", bufs=2, space="DRAM")
            agoutp = tc.alloc_tile_pool(name="agout", bufs=2, space="DRAM")
            g0ps = tc.alloc_tile_pool(name="g0psum", bufs=1, space="PSUM")
            g1ps = tc.alloc_tile_pool(name="g1psum", bufs=1, space="PSUM")
            fcps = tc.alloc_tile_pool(name="fcpsum", bufs=2, space="PSUM")
            dups = tc.alloc_tile_pool(name="dumpsum", bufs=1, space="PSUM")

            # critical-path pre-loop loads first on the sync ring
            featp = tc.alloc_tile_pool(name="feat", bufs=1)
            initwp = tc.alloc_tile_pool(name="initw", bufs=1)
            featT_s = featp.tile([P, KF, B], F16)
            nc.sync.dma_start(featT_s, featT_v)
            initw_s = initwp.tile([P, KF, 4 * P], F16)
            nc.sync.dma_start(initw_s, initw_v)

            id128 = constp.tile([P, P], F16)
            make_identity(nc, id128)
            b0T_s = constp.tile([P, 4], F32, tag="b0T")
            nc.sync.dma_start(b0T_s, b0T)
            b1row_s = constp.tile([1, Gc], F16, tag="b1row")
            nc.sync.dma_start(b1row_s, b1row)
            ones1 = constp.tile([1, B], F16, tag="ones1")
            nc.gpsimd.memset(ones1, 1.0)
            initbT_s = constp.tile([P, 4], F32, tag="ibT")
            nc.sync.dma_start(initbT_s, initbT)
            idx_s = constp.tile([P, T, 1], mybir.dt.int32, tag="idx")
            nc.sync.dma_start(idx_s, idx_v)

            # ---- resident weights (scalar-engine DMA ring) ---------------
            whh0_s = wresp.tile([P, KH, Gc], F16, tag="whh0")
            nc.scalar.dma_start(whh0_s, whh0T_v)
            wih1_s = wresp.tile([P, KH, Gc], F16, tag="wih1")
            nc.scalar.dma_start(wih1_s, wih1T_v)
            whh1_s = wresp.tile([P, KH, Gc], F16, tag="whh1")
            nc.scalar.dma_start(whh1_s, whh1T_v)
            wih0_s = wresp.tile([P, KE, Gc], F16, tag="wih0")
            nc.scalar.dma_start(wih0_s, wih0T_v)
            fcw_s = wresp.tile([P, KH, Vc], F16, tag="fcw")
            nc.scalar.dma_start(fcw_s, fcwT_v)
            fcb_s = wresp.tile([P, Vc], F32, tag="fcb")
            nc.scalar.dma_start(fcb_s, fcb_rep)

            # persistent state
            c0T_s = statep.tile([P, B], F32, tag="c0")
            c1T_s = statep.tile([P, B], F32, tag="c1")

            x0T_s = x0p.tile([P, 4, T, B], F16)
            dum_ps = dups.tile([1, 512], F32)

            # ---- phase A-gather helper: Q7 desc-gen is ~1.1us per call,
            # so gathers are interleaved between collective emissions to
            # keep them off the doorbell path
            row_tiles = []

            def gather(m):
                assert m == len(row_tiles)
                rows = rowsp.tile([P, E], F16, tag="rows")
                nc.gpsimd.indirect_dma_start(
                    out=rows[:],
                    out_offset=None,
                    in_=table[:],
                    in_offset=bass.IndirectOffsetOnAxis(ap=idx_s[:, m, :], axis=0),
                )
                row_tiles.append(rows)

            # ---- phase B: h/c init (linear head, orientation B) ----------
            initps = tc.alloc_tile_pool(name="initpsum", bufs=1, space="PSUM")
            ips = initps.tile([P, 4, P], F32)
            for m in range(4):
                for k in range(KF):
                    nc.tensor.matmul(
                        ips[:, m, :],
                        initw_s[:, k, m * P : (m + 1) * P],
                        featT_s[:, k, :],
                        start=(k == 0),
                        stop=(k == KF - 1),
                    )
            hpair_init = hpairp.tile([P, 2 * B], F16, tag="hpair")
            nc.vector.tensor_scalar_add(hpair_init[:, 0:B], ips[:, 0, :],
                                        initbT_s[:, 0:1])
            nc.vector.tensor_scalar_add(hpair_init[:, B : 2 * B], ips[:, 1, :],
                                        initbT_s[:, 1:2])
            nc.vector.tensor_scalar_add(c0T_s, ips[:, 2, :],
                                        initbT_s[:, 2:3])
            nc.vector.tensor_scalar_add(c1T_s, ips[:, 3, :],
                                        initbT_s[:, 3:4])

            # ---- AG_init = {h0_init, h1_init} ----------------------------
            agin_t = aginp.tile([P, 2 * B], F16, tag="agin")
            agout_t = agoutp.tile([NCORES * P, 2 * B], F16, tag="agout",
                                  addr_space="Shared")
            nc.sync.dma_start(agin_t, hpair_init)
            nc.gpsimd.collective_compute(
                "AllGather", mybir.AluOpType.bypass, replica_groups=RG,
                ins=[agin_t[:].opt()], outs=[agout_t[:].opt()],
            )
            agout_prev = agout_t

            for m in range(10):
                gather(m)

            # ---- phase A-compute: embT transpose + X0T for chunk m -------
            initps.release()
            embps = tc.alloc_tile_pool(name="embpsum", bufs=2, space="PSUM")
            x0ps = tc.alloc_tile_pool(name="x0psum", bufs=1, space="PSUM")
            embcp = tc.alloc_tile_pool(name="embc", bufs=2)

            def a_compute(m):
                pt = embps.tile([P, KE, P], F16, tag="pt")
                for ke in range(KE):
                    nc.tensor.transpose(
                        pt[:, ke, :],
                        row_tiles[m][:, ke * P : (ke + 1) * P],
                        id128,
                    )
                embc = embcp.tile([P, KE, P], F16, tag="embc")
                nc.vector.tensor_copy(embc, pt)
                xps = x0ps.tile([P, 4, P], F32, tag="xps")
                for g in range(4):
                    for k in range(KE):
                        nc.tensor.matmul(
                            xps[:, g, :],
                            wih0_s[:, k, g * P : (g + 1) * P],
                            embc[:, k, :],
                            start=(k == 0),
                            stop=(k == KE - 1),
                        )
                for g in range(4):
                    nc.vector.tensor_scalar_add(
                        x0T_s[:, g, m, :], xps[:, g, :], b0T_s[:, g : g + 1]
                    )

            for m in range(4):
                a_compute(m)

            # ---- gate drains (orientation B; biases already in psum) -----
            def drain(gps, cT_s, out_hT):
                """gatesT [P,4,B] psum -> hT f16 into out_hT; updates cT_s."""
                sig_ifo = gactp.tile([P, 3, B], F32, tag="sig_ifo")
                nc.scalar.activation(sig_ifo, gps[:, 0:3, :], SIG)
                tanh_g = gactp.tile([P, B], F32, tag="tanh_g")
                nc.scalar.activation(tanh_g, gps[:, 3, :], TANH)
                nc.vector.tensor_mul(cT_s, sig_ifo[:, 1, :], cT_s)
                nc.vector.tensor_mul(tanh_g, sig_ifo[:, 0, :], tanh_g)
                nc.vector.tensor_add(cT_s, cT_s, tanh_g)
                tanh_c = gactp.tile([P, B], F32, tag="tanh_c")
                nc.scalar.activation(tanh_c, cT_s, TANH)
                nc.vector.tensor_mul(out_hT, sig_ifo[:, 2, :], tanh_c)

            # ---- main loop -----------------------------------------------
            hpair_prev = hpair_init
            for t in range(T + 2):
                # readback of AG_{t-1} (for t==0: AG_init)
                rbv = agout_prev[:].rearrange("(c p) x -> p c x", c=NCORES)
                R = rbp.tile([P, NCORES, 2 * B], F16, tag="R")
                nc.sync.dma_start(R[:, 0:4, :], rbv[:, 0:4, :])
                nc.scalar.dma_start(R[:, 4:NCORES, :], rbv[:, 4:NCORES, :])

                hpair_t = hpairp.tile([P, 2 * B], F16, tag="hpair")
                agin_t = agout_t = None
                if t <= T:
                    agin_t = aginp.tile([P, 2 * B], F16, tag="agin")
                    agout_t = agoutp.tile([NCORES * P, 2 * B], F16,
                                          tag="agout", addr_space="Shared")

                # ---- L0(t): gates0T = whh0.h0(t-1) + X0T[t] --------------
                if t < T:
                    gps0 = g0ps.tile([P, 4, B], F32, tag="g0")
                    for g in range(4):
                        for k in range(KH):
                            nc.tensor.matmul(
                                gps0[:, g, :],
                                whh0_s[:, k, g * P : (g + 1) * P],
                                R[:, k, 0:B],
                                start=(k == 0),
                                stop=False,
                            )
                        nc.tensor.matmul(
                            gps0[:, g, :],
                            id128,
                            x0T_s[:, g, t, :],
                            start=False,
                            stop=True,
                        )
                    drain(gps0, c0T_s, hpair_t[:, 0:B])
                else:
                    nc.vector.tensor_copy(hpair_t[:, 0:B], hpair_prev[:, 0:B])
                if agin_t is not None:
                    nc.scalar.dma_start(agin_t[:, 0:B], hpair_t[:, 0:B])

                # ---- L1(t-1): gates1T = wih1.h0(t-1) + whh1.h1(t-2) ------
                if 1 <= t <= T:
                    gps1 = g1ps.tile([P, 4, B], F32, tag="g1")
                    for g in range(4):
                        for k in range(KH):
                            nc.tensor.matmul(
                                gps1[:, g, :],
                                wih1_s[:, k, g * P : (g + 1) * P],
                                R[:, k, 0:B],
                                start=(k == 0),
                                stop=False,
                            )
                        for k in range(KH):
                            nc.tensor.matmul(
                                gps1[:, g, :],
                                whh1_s[:, k, g * P : (g + 1) * P],
                                R[:, k, B : 2 * B],
                                start=False,
                                stop=False,
                            )
                        nc.tensor.matmul(
                            gps1[:, g, :],
                            b1row_s[:, g * P : (g + 1) * P],
                            ones1,
                            start=False,
                            stop=True,
                        )
                    drain(gps1, c1T_s, hpair_t[:, B : 2 * B])
                elif t == 0:
                    nc.vector.tensor_copy(hpair_t[:, B : 2 * B],
                                          hpair_prev[:, B : 2 * B])

                # ---- AG_t = {h0T(t), h1T(t-1)} ---------------------------
                if t <= T:
                    nc.sync.dma_start(agin_t[:, B : 2 * B],
                                      hpair_t[:, B : 2 * B])
                    nc.gpsimd.collective_compute(
                        "AllGather", mybir.AluOpType.bypass,
                        replica_groups=RG,
                        ins=[agin_t[:].opt()], outs=[agout_t[:].opt()],
                    )
                    agout_prev = agout_t

                # ---- interleave remaining gather desc-gens ---------------
                while len(row_tiles) < T and len(row_tiles) < 10 + 5 * (t + 1):
                    gather(len(row_tiles))

                # ---- FC(t-2) on R part1 = h1T_full(t-2) ------------------
                if t >= 2:
                    tau = t - 2
                    for j0 in range(0, Vc, 512):
                        w = min(512, Vc - j0)
                        fps = fcps.tile([P, 512], F32, tag="fc")
                        for k in range(KH):
                            nc.tensor.matmul(
                                fps[:, :w],
                                R[:, k, B : 2 * B],
                                fcw_s[:, k, j0 : j0 + w],
                                start=(k == 0),
                                stop=(k == KH - 1),
                            )
                        ot = fcoutp.tile([P, 512], F32, tag="ot")
                        nc.vector.tensor_add(
                            ot[:, :w], fps[:, :w], fcb_s[:, j0 : j0 + w]
                        )
                        nc.scalar.dma_start(
                            out[tau * P : (tau + 1) * P, j0 : j0 + w],
                            ot[:, :w],
                        )

                # ---- A-compute filler for a later chunk ------------------
                if t + 4 < T:
                    a_compute(t + 4)

                # ---- PE warm-keepers during the collective wait ----------
                if t <= T:
                    for _ in range(NDUMMY):
                        nc.tensor.matmul(
                            dum_ps, id128[:, 0:1], whh0_s[:, 0, :],
                            start=True, stop=True,
                        )

                hpair_prev = hpair_t

            for pool in (embcp, x0ps, embps, initwp, featp, dups, fcps,
                         g1ps, g0ps, agoutp, aginp, fcoutp, gactp, hpairp,
                         rbp, rowsp, x0p, statep, wresp, constp):
                pool.release()

    nc.finalize()
    return nc


def _get_compiled():
    if "nc" not in _cache:
        _cache["nc"] = _build_nc()
    return _cache["nc"]


def _prep_inputs(features, captions, embed_table, init_h_w, init_h_b,
                 init_c_w, init_c_b, w_ih0, w_hh0, b_ih0, b_hh0,
                 w_ih1, w_hh1, b_ih1, b_hh1, fc_w, fc_b):
    f16 = lambda x: np.ascontiguousarray(np.asarray(x, dtype=np.float32)).astype(np.float16)
    f32 = lambda x: np.ascontiguousarray(np.asarray(x, dtype=np.float32))

    features = np.asarray(features, dtype=np.float32)
    captions = np.asarray(captions).astype(np.int32)

    shared = {
        "featT": f16(features.T),
        "table": f16(embed_table),
        # row r = t*B + b  ->  captions[b, t]
        "emb_idx": np.ascontiguousarray(captions.T.reshape(TB, 1)),
    }

    # torch gate order i,f,g,o -> local order [i, f, o, g]
    def gate_rows(c):
        base = np.arange(c * Hc, (c + 1) * Hc)
        return np.concatenate([base, H + base, 3 * H + base, 2 * H + base])

    def init_sel(c):
        # Linear output col r maps to (h = r // L, l = r % L)
        h_idx = np.arange(c * Hc, (c + 1) * Hc)
        return 2 * h_idx, 2 * h_idx + 1   # l0 rows, l1 rows

    in_maps = []
    for c in range(NCORES):
        rows_sel = gate_rows(c)
        l0, l1 = init_sel(c)
        ihw = np.asarray(init_h_w, np.float32)
        icw = np.asarray(init_c_w, np.float32)
        ihb = np.asarray(init_h_b, np.float32)
        icb = np.asarray(init_c_b, np.float32)
        initw = np.concatenate([ihw[l0], ihw[l1], icw[l0], icw[l1]], axis=0)
        initb = np.concatenate([ihb[l0], ihb[l1], icb[l0], icb[l1]])

        b0 = (np.asarray(b_ih0, np.float32) + np.asarray(b_hh0, np.float32))[rows_sel]
        b1 = (np.asarray(b_ih1, np.float32) + np.asarray(b_hh1, np.float32))[rows_sel]

        vsl = slice(c * Vc, (c + 1) * Vc)
        m = dict(shared)
        m.update({
            "initw": f16(initw.T),
            "initbT": f32(initb.reshape(4, P).T),
            "wih0T": f16(np.asarray(w_ih0, np.float32)[rows_sel].T),
            "whh0T": f16(np.asarray(w_hh0, np.float32)[rows_sel].T),
            "wih1T": f16(np.asarray(w_ih1, np.float32)[rows_sel].T),
            "whh1T": f16(np.asarray(w_hh1, np.float32)[rows_sel].T),
            "b0T": f32(b0.reshape(4, P).T),
            "b1row": f16(b1.reshape(1, Gc)),
            "fcwT": f16(np.asarray(fc_w, np.float32)[vsl].T),
            "fcb_rep": f32(np.broadcast_to(
                np.asarray(fc_b, np.float32)[vsl], (P, Vc))),
        })
        in_maps.append(m)
    return in_maps


last_results = None


def kernel(**inputs) -> np.ndarray:
    global last_results
    nc = _get_compiled()
    in_maps = _prep_inputs(**inputs)
    res = run_bass_kernel_spmd(nc, in_maps, core_ids=list(range(NCORES)))
    last_results = res
    parts = [res.results[c]["out"].reshape(T, B, Vc) for c in range(NCORES)]
    return np.concatenate(parts, axis=2)


# revision 5
# speedup vs baseline: 1.0999x; 1.0454x over previous
"""Trainium2 Bass kernel for a 2-layer LSTM decoder (B=128, T=32, F=2048,
E=512, H=1024, V=10000), tensor-parallel over the hidden dim across 8
NeuronCores.

Sharding: core c owns hidden slice [c*128, (c+1)*128) of BOTH layers (gates
i,f,o,g for that slice = 512 gate rows per weight matrix) and vocab slice
[c*1250, (c+1)*1250) of the FC head. Full batch B=128 on every core, so
every recurrence matmul runs at full 128-wide PE utilization; the per-step
cost is one 64KB AllGather of the {h0T | h1T} slice pair (DRAM-bounce
collective, ~7us ncfw floor) whose latency is hidden behind the FC matmuls
of older timesteps.

All gate math is "orientation B" (transposed): gatesT[g, b] tiles with the
gate index on partitions, so h-slices come out of the elementwise tail
already transposed for the next step's lhsT and for the AllGather, and the
gate biases fold into the ACT bias operand (no bias matmuls).

AG payload rows are 512B ({h0T | h1T} side by side per partition) so the
readback is 512B-descriptor DMA, and a small batch of throwaway matmuls at
the end of each iteration keeps the PE HAM clock-gate at 2.4 GHz across the
collective wait.

Per-step dependency chain (iteration t emits):
  readback R of AG_{t-1}={h0(t-1), h1(t-2)} -> L0(t) -> h0T(t)
  L1(t-1) from R -> h1T(t-1);  AG_t = {h0T(t), h1T(t-1)}
  FC(t-2) from R part1 (per-step PE filler that covers AG latency)
"""

import numpy as np

import concourse.bass as bass
import concourse.mybir as mybir
from concourse import bacc
from concourse.bass_utils import run_bass_kernel_spmd
from concourse.masks import make_identity
from concourse.tile import TileContext

P = 128
NCORES = 8
B, T, F, E, H, L, V = 128, 32, 2048, 512, 1024, 2, 10000
G = 4 * H
TB = T * B                 # 4096 (t, b) rows
Hc = H // NCORES           # 128 hidden units per core
Gc = 4 * Hc                # 512 local gate rows
Vc = V // NCORES           # 1250 vocab cols per core
KE, KF, KH = E // P, F // P, H // P   # 4, 16, 8
NDUMMY = 0                # PE warm-keeper matmuls per iteration
F16 = mybir.dt.float16
F32 = mybir.dt.float32

_cache = {}

SIG = mybir.ActivationFunctionType.Sigmoid
TANH = mybir.ActivationFunctionType.Tanh


def _build_nc():
    nc = bacc.Bacc("TRN2", target_bir_lowering=False, debug=False,
                   enable_asserts=False, num_devices=NCORES)

    def din(name, shape, dt=F16):
        return nc.dram_tensor(name, shape, dt, kind="ExternalInput").ap()

    featT = din("featT", [F, B])
    emb_idx = din("emb_idx", [TB, 1], mybir.dt.int32)
    table = din("table", [V, E])
    initw = din("initw", [F, 4 * P])      # cols: h_l0 | h_l1 | c_l0 | c_l1
    initbT = din("initbT", [P, 4], F32)
    wih0T = din("wih0T", [E, Gc])
    whh0T = din("whh0T", [H, Gc])
    wih1T = din("wih1T", [H, Gc])
    whh1T = din("whh1T", [H, Gc])
    b0T = din("b0T", [P, 4], F32)
    b1row = din("b1row", [1, Gc])
    fcwT = din("fcwT", [H, Vc])
    fcb_rep = din("fcb_rep", [P, Vc], F32)

    out = nc.dram_tensor("out", [TB, Vc], F32, kind="ExternalOutput").ap()

    featT_v = featT.rearrange("(k p) b -> p k b", p=P)
    initw_v = initw.rearrange("(k p) n -> p k n", p=P)
    wih0T_v = wih0T.rearrange("(k p) g -> p k g", p=P)
    whh0T_v = whh0T.rearrange("(k p) g -> p k g", p=P)
    wih1T_v = wih1T.rearrange("(k p) g -> p k g", p=P)
    whh1T_v = whh1T.rearrange("(k p) g -> p k g", p=P)
    fcwT_v = fcwT.rearrange("(k p) v -> p k v", p=P)
    idx_v = emb_idx.rearrange("(g p) one -> p g one", p=P)

    RG = [list(range(NCORES))]

    with TileContext(nc) as tc:
        if True:
            constp = tc.alloc_tile_pool(name="const", bufs=1)
            wresp = tc.alloc_tile_pool(name="wres", bufs=1)
            statep = tc.alloc_tile_pool(name="state", bufs=1)
            x0p = tc.alloc_tile_pool(name="x0", bufs=1)
            rowsp = tc.alloc_tile_pool(name="rows", bufs=32)
            rbp = tc.alloc_tile_pool(name="rb", bufs=2)
            hpairp = tc.alloc_tile_pool(name="hpair", bufs=2)
            gactp = tc.alloc_tile_pool(name="gact", bufs=2)
            fcoutp = tc.alloc_tile_pool(name="fcout", bufs=3)
            aginp = tc.alloc_tile_pool(name="agin", bufs=2, space="DRAM")
            agoutp = tc.alloc_tile_pool(name="agout", bufs=2, space="DRAM")
            g0ps = tc.alloc_tile_pool(name="g0psum", bufs=1, space="PSUM")
            g1ps = tc.alloc_tile_pool(name="g1psum", bufs=1, space="PSUM")
            fcps = tc.alloc_tile_pool(name="fcpsum", bufs=2, space="PSUM")
            dups = tc.alloc_tile_pool(name="dumpsum", bufs=1, space="PSUM")

            # critical-path pre-loop loads first on the sync ring
            featp = tc.alloc_tile_pool(name="feat", bufs=1)
            initwp = tc.alloc_tile_pool(name="initw", bufs=1)
            featT_s = featp.tile([P, KF, B], F16)
            nc.sync.dma_start(featT_s, featT_v)
            initw_s = initwp.tile([P, KF, 4 * P], F16)
            nc.sync.dma_start(initw_s, initw_v)

            id128 = constp.tile([P, P], F16)
            make_identity(nc, id128)
            b0T_s = constp.tile([P, 4], F32, tag="b0T")
            nc.sync.dma_start(b0T_s, b0T)
            b1row_s = constp.tile([1, Gc], F16, tag="b1row")
            nc.sync.dma_start(b1row_s, b1row)
            ones1 = constp.tile([1, B], F16, tag="ones1")
            nc.gpsimd.memset(ones1, 1.0)
            initbT_s = constp.tile([P, 4], F32, tag="ibT")
            nc.sync.dma_start(initbT_s, initbT)
            idx_s = constp.tile([P, T, 1], mybir.dt.int32, tag="idx")
            nc.sync.dma_start(idx_s, idx_v)

            # ---- resident weights (scalar-engine DMA ring) ---------------
            whh0_s = wresp.tile([P, KH, Gc], F16, tag="whh0")
            nc.scalar.dma_start(whh0_s, whh0T_v)
            wih1_s = wresp.tile([P, KH, Gc], F16, tag="wih1")
            nc.scalar.dma_start(wih1_s, wih1T_v)
            whh1_s = wresp.tile([P, KH, Gc], F16, tag="whh1")
            nc.scalar.dma_start(whh1_s, whh1T_v)
            wih0_s = wresp.tile([P, KE, Gc], F16, tag="wih0")
            nc.scalar.dma_start(wih0_s, wih0T_v)
            fcw_s = wresp.tile([P, KH, Vc], F16, tag="fcw")
            nc.scalar.dma_start(fcw_s, fcwT_v)
            fcb_s = wresp.tile([P, Vc], F32, tag="fcb")
            nc.scalar.dma_start(fcb_s, fcb_rep)

            # persistent state
            c0T_s = statep.tile([P, B], F32, tag="c0")
            c1T_s = statep.tile([P, B], F32, tag="c1")

            x0T_s = x0p.tile([P, 4, T, B], F16)
            dum_ps = dups.tile([1, 512], F32)

            # ---- phase A-gather helper: Q7 desc-gen is ~1.1us per call,
            # so gathers are interleaved between collective emissions to
            # keep them off the doorbell path
            row_tiles = []

            def gather(m):
                assert m == len(row_tiles)
                rows = rowsp.tile([P, E], F16, tag="rows")
                nc.gpsimd.indirect_dma_start(
                    out=rows[:],
                    out_offset=None,
                    in_=table[:],
                    in_offset=bass.IndirectOffsetOnAxis(ap=idx_s[:, m, :], axis=0),
                )
                row_tiles.append(rows)

            for m in range(16):
                gather(m)

            # ---- phase B: h/c init (linear head, orientation B) ----------
            initps = tc.alloc_tile_pool(name="initpsum", bufs=1, space="PSUM")
            ips = initps.tile([P, 4, P], F32)
            for m in range(4):
                for k in range(KF):
                    nc.tensor.matmul(
                        ips[:, m, :],
                        initw_s[:, k, m * P : (m + 1) * P],
                        featT_s[:, k, :],
                        start=(k == 0),
                        stop=(k == KF - 1),
                    )
            hpair_init = hpairp.tile([P, 2 * B], F16, tag="hpair")
            nc.vector.tensor_scalar_add(hpair_init[:, 0:B], ips[:, 0, :],
                                        initbT_s[:, 0:1])
            nc.vector.tensor_scalar_add(hpair_init[:, B : 2 * B], ips[:, 1, :],
                                        initbT_s[:, 1:2])
            nc.vector.tensor_scalar_add(c0T_s, ips[:, 2, :],
                                        initbT_s[:, 2:3])
            nc.vector.tensor_scalar_add(c1T_s, ips[:, 3, :],
                                        initbT_s[:, 3:4])

            # ---- AG_init = {h0_init, h1_init} ----------------------------
            agin_t = aginp.tile([P, 2 * B], F16, tag="agin")
            agout_t = agoutp.tile([NCORES * P, 2 * B], F16, tag="agout",
                                  addr_space="Shared")
            nc.sync.dma_start(agin_t, hpair_init)
            nc.gpsimd.collective_compute(
                "AllGather", mybir.AluOpType.bypass, replica_groups=RG,
                ins=[agin_t[:].opt()], outs=[agout_t[:].opt()],
            )
            agout_prev = agout_t

            for m in range(16, T):
                gather(m)

            # ---- phase A-compute: embT transpose + X0T for chunk m -------
            initps.release()
            embps = tc.alloc_tile_pool(name="embpsum", bufs=2, space="PSUM")
            x0ps = tc.alloc_tile_pool(name="x0psum", bufs=1, space="PSUM")
            embcp = tc.alloc_tile_pool(name="embc", bufs=2)

            def a_compute(m):
                pt = embps.tile([P, KE, P], F16, tag="pt")
                for ke in range(KE):
                    nc.tensor.transpose(
                        pt[:, ke, :],
                        row_tiles[m][:, ke * P : (ke + 1) * P],
                        id128,
                    )
                embc = embcp.tile([P, KE, P], F16, tag="embc")
                nc.vector.tensor_copy(embc, pt)
                xps = x0ps.tile([P, 4, P], F32, tag="xps")
                for g in range(4):
                    for k in range(KE):
                        nc.tensor.matmul(
                            xps[:, g, :],
                            wih0_s[:, k, g * P : (g + 1) * P],
                            embc[:, k, :],
                            start=(k == 0),
                            stop=(k == KE - 1),
                        )
                for g in range(4):
                    nc.vector.tensor_scalar_add(
                        x0T_s[:, g, m, :], xps[:, g, :], b0T_s[:, g : g + 1]
                    )

            for m in range(4):
                a_compute(m)

            # ---- gate drains (orientation B; biases already in psum) -----
            def drain(gps, cT_s, out_hT):
                """gatesT [P,4,B] psum -> hT f16 into out_hT; updates cT_s."""
                sig_ifo = gactp.tile([P, 3, B], F32, tag="sig_ifo")
                nc.scalar.activation(sig_ifo, gps[:, 0:3, :], SIG)
                tanh_g = gactp.tile([P, B], F32, tag="tanh_g")
                nc.scalar.activation(tanh_g, gps[:, 3, :], TANH)
                nc.vector.tensor_mul(cT_s, sig_ifo[:, 1, :], cT_s)
                nc.vector.tensor_mul(tanh_g, sig_ifo[:, 0, :], tanh_g)
                nc.vector.tensor_add(cT_s, cT_s, tanh_g)
                tanh_c = gactp.tile([P, B], F32, tag="tanh_c")
                nc.scalar.activation(tanh_c, cT_s, TANH)
                nc.vector.tensor_mul(out_hT, sig_ifo[:, 2, :], tanh_c)

            # ---- main loop -----------------------------------------------
            hpair_prev = hpair_init
            for t in range(T + 2):
                # readback of AG_{t-1} (for t==0: AG_init)
                rbv = agout_prev[:].rearrange("(c p) x -> p c x", c=NCORES)
                R = rbp.tile([P, NCORES, 2 * B], F16, tag="R")
                nc.sync.dma_start(R[:, 0:4, :], rbv[:, 0:4, :])
                nc.scalar.dma_start(R[:, 4:NCORES, :], rbv[:, 4:NCORES, :])

                hpair_t = hpairp.tile([P, 2 * B], F16, tag="hpair")
                agin_t = agout_t = None
                if t <= T:
                    agin_t = aginp.tile([P, 2 * B], F16, tag="agin")
                    agout_t = agoutp.tile([NCORES * P, 2 * B], F16,
                                          tag="agout", addr_space="Shared")

                # ---- L0(t): gates0T = whh0.h0(t-1) + X0T[t] --------------
                if t < T:
                    gps0 = g0ps.tile([P, 4, B], F32, tag="g0")
                    for g in range(4):
                        for k in range(KH):
                            nc.tensor.matmul(
                                gps0[:, g, :],
                                whh0_s[:, k, g * P : (g + 1) * P],
                                R[:, k, 0:B],
                                start=(k == 0),
                                stop=False,
                            )
                        nc.tensor.matmul(
                            gps0[:, g, :],
                            id128,
                            x0T_s[:, g, t, :],
                            start=False,
                            stop=True,
                        )
                    drain(gps0, c0T_s, hpair_t[:, 0:B])
                else:
                    nc.vector.tensor_copy(hpair_t[:, 0:B], hpair_prev[:, 0:B])
                if agin_t is not None:
                    nc.scalar.dma_start(agin_t[:, 0:B], hpair_t[:, 0:B])

                # ---- L1(t-1): gates1T = wih1.h0(t-1) + whh1.h1(t-2) ------
                if 1 <= t <= T:
                    gps1 = g1ps.tile([P, 4, B], F32, tag="g1")
                    for g in range(4):
                        for k in range(KH):
                            nc.tensor.matmul(
                                gps1[:, g, :],
                                wih1_s[:, k, g * P : (g + 1) * P],
                                R[:, k, 0:B],
                                start=(k == 0),
                                stop=False,
                            )
                        for k in range(KH):
                            nc.tensor.matmul(
                                gps1[:, g, :],
                                whh1_s[:, k, g * P : (g + 1) * P],
                                R[:, k, B : 2 * B],
                                start=False,
                                stop=False,
                            )
                        nc.tensor.matmul(
                            gps1[:, g, :],
                            b1row_s[:, g * P : (g + 1) * P],
                            ones1,
                            start=False,
                            stop=True,
                        )
                    drain(gps1, c1T_s, hpair_t[:, B : 2 * B])
                elif t == 0:
                    nc.vector.tensor_copy(hpair_t[:, B : 2 * B],
                                          hpair_prev[:, B : 2 * B])

                # ---- AG_t = {h0T(t), h1T(t-1)} ---------------------------
                if t <= T:
                    nc.sync.dma_start(agin_t[:, B : 2 * B],
                                      hpair_t[:, B : 2 * B])
                    nc.gpsimd.collective_compute(
                        "AllGather", mybir.AluOpType.bypass,
                        replica_groups=RG,
                        ins=[agin_t[:].opt()], outs=[agout_t[:].opt()],
                    )
                    agout_prev = agout_t

                # ---- FC(t-2) on R part1 = h1T_full(t-2) ------------------
                if t >= 2:
                    tau = t - 2
                    for j0 in range(0, 1024, 512):
                        w = min(512, Vc - j0)
                        fps = fcps.tile([P, 512], F32, tag="fc")
                        for k in range(KH):
                            nc.tensor.matmul(
                                fps[:, :w],
                                R[:, k, B : 2 * B],
                                fcw_s[:, k, j0 : j0 + w],
                                start=(k == 0),
                                stop=(k == KH - 1),
                            )
                        ot = fcoutp.tile([P, 512], F32, tag="ot")
                        nc.vector.tensor_add(
                            ot[:, :w], fps[:, :w], fcb_s[:, j0 : j0 + w]
                        )
                        nc.scalar.dma_start(
                            out[tau * P : (tau + 1) * P, j0 : j0 + w],
                            ot[:, :w],
                        )

                # ---- A-compute filler for a later chunk ------------------
                if t + 4 < T:
                    a_compute(t + 4)

                # ---- FC tail chunk: PE filler deep in the collective wait
                if t >= 2:
                    tau = t - 2
                    j0 = 1024
                    w = Vc - j0
                    fps = fcps.tile([P, 512], F32, tag="fc")
                    for k in range(KH):
                        nc.tensor.matmul(
                            fps[:, :w],
                            R[:, k, B : 2 * B],
                            fcw_s[:, k, j0 : j0 + w],
                            start=(k == 0),
                            stop=(k == KH - 1),
                        )
                    ot = fcoutp.tile([P, 512], F32, tag="ot")
                    nc.vector.tensor_add(
                        ot[:, :w], fps[:, :w], fcb_s[:, j0 : j0 + w]
                    )
                    nc.scalar.dma_start(
                        out[tau * P : (tau + 1) * P, j0 : j0 + w],
                        ot[:, :w],
                    )

                # ---- PE warm-keepers during the collective wait ----------
                if t <= T:
                    for _ in range(NDUMMY):
                        nc.tensor.matmul(
                            dum_ps, id128[:, 0:1], whh0_s[:, 0, :],
                            start=True, stop=True,
                        )

                hpair_prev = hpair_t

            for pool in (embcp, x0ps, embps, initwp, featp, dups, fcps,
                         g1ps, g0ps, agoutp, aginp, fcoutp, gactp, hpairp,
                         rbp, rowsp, x0p, statep, wresp, constp):
                pool.release()

    nc.finalize()
    return nc


def _get_compiled():
    if "nc" not in _cache:
        _cache["nc"] = _build_nc()
    return _cache["nc"]


def _prep_inputs(features, captions, embed_table, init_h_w, init_h_b,
                 init_c_w, init_c_b, w_ih0, w_hh0, b_ih0, b_hh0,
                 w_ih1, w_hh1, b_ih1, b_hh1, fc_w, fc_b):
    f16 = lambda x: np.ascontiguousarray(np.asarray(x, dtype=np.float32)).astype(np.float16)
    f32 = lambda x: np.ascontiguousarray(np.asarray(x, dtype=np.float32))

    features = np.asarray(features, dtype=np.float32)
    captions = np.asarray(captions).astype(np.int32)

    shared = {
        "featT": f16(features.T),
        "table": f16(embed_table),
        # row r = t*B + b  ->  captions[b, t]
        "emb_idx": np.ascontiguousarray(captions.T.reshape(TB, 1)),
    }

    # torch gate order i,f,g,o -> local order [i, f, o, g]
    def gate_rows(c):
        base = np.arange(c * Hc, (c + 1) * Hc)
        return np.concatenate([base, H + base, 3 * H + base, 2 * H + base])

    def init_sel(c):
        # Linear output col r maps to (h = r // L, l = r % L)
        h_idx = np.arange(c * Hc, (c + 1) * Hc)
        return 2 * h_idx, 2 * h_idx + 1   # l0 rows, l1 rows

    in_maps = []
    for c in range(NCORES):
        rows_sel = gate_rows(c)
        l0, l1 = init_sel(c)
        ihw = np.asarray(init_h_w, np.float32)
        icw = np.asarray(init_c_w, np.float32)
        ihb = np.asarray(init_h_b, np.float32)
        icb = np.asarray(init_c_b, np.float32)
        initw = np.concatenate([ihw[l0], ihw[l1], icw[l0], icw[l1]], axis=0)
        initb = np.concatenate([ihb[l0], ihb[l1], icb[l0], icb[l1]])

        b0 = (np.asarray(b_ih0, np.float32) + np.asarray(b_hh0, np.float32))[rows_sel]
        b1 = (np.asarray(b_ih1, np.float32) + np.asarray(b_hh1, np.float32))[rows_sel]

        vsl = slice(c * Vc, (c + 1) * Vc)
        m = dict(shared)
        m.update({
            "initw": f16(initw.T),
            "initbT": f32(initb.reshape(4, P).T),
            "wih0T": f16(np.asarray(w_ih0, np.float32)[rows_sel].T),
            "whh0T": f16(np.asarray(w_hh0, np.float32)[rows_sel].T),
            "wih1T": f16(np.asarray(w_ih1, np.float32)[rows_sel].T),
            "whh1T": f16(np.asarray(w_hh1, np.float32)[rows_sel].T),
            "b0T": f32(b0.reshape(4, P).T),
            "b1row": f16(b1.reshape(1, Gc)),
            "fcwT": f16(np.asarray(fc_w, np.float32)[vsl].T),
            "fcb_rep": f32(np.broadcast_to(
                np.asarray(fc_b, np.float32)[vsl], (P, Vc))),
        })
        in_maps.append(m)
    return in_maps


last_results = None


def kernel(**inputs) -> np.ndarray:
    global last_results
    nc = _get_compiled()
    in_maps = _prep_inputs(**inputs)
    res = run_bass_kernel_spmd(nc, in_maps, core_ids=list(range(NCORES)))
    last_results = res
    parts = [res.results[c]["out"].reshape(T, B, Vc) for c in range(NCORES)]
    return np.concatenate(parts, axis=2)
